# revision 26
# baseline (speedup 1.0000x reference)
"""Trainium2 Bass kernel for CEN patch expert (im2col + patch-norm + 122-512-128-1 MLP).

Strategy (8 NeuronCores, data-parallel over batch B=32 -> 4 images/core):
  - Patch stats computed separably (vertical band-matmul + horizontal
    log-shift sliding sums), normalization folded into MM1 contraction rows
    (rhs rows = [p*inv (121); mean*inv; std*inv], lhsT = [Wp.T; -rowsum;
    W1[:,0]+b1]).
  - bf16 datapath for patches/weights/activations (rel-err budget 2e-2).
  - ScalarE (tanh) is the bottleneck. Per tile, ONE activation covers
    [z1 (4x256) | z2-left-half (128) of tile g-2] contiguously in the slot
    (slot layout [z1 | z2]); the z2 RIGHT half (128 cols) is offloaded to
    DVE via a clamped odd-quintic tanh approximation (Horner, bf16 ops,
    pairs of tiles batched to amortize the PSUM read).
  - z3 computed TRANSPOSED on the PE: per 128-position block, one [128,1]
    matmul (lhsT = h2 slice) into a per-image [128,74] PSUM block; ONE
    sigmoid per image + PE transpose + copy + 2 output DMAs. No per-tile
    z3 copies on the DVE. mm3t goes FIRST in each iter's PE block and the
    h pool is 6 deep so tanh never transitively waits a queued mm3t.
  - Patch-std via ALU-only rsqrt (bit-trick + 1 Newton step): no ScalarE
    Sqrt, so tanh+sigmoid live in one activation table; a dummy sigmoid at
    t=0 preloads that table during startup idle (zero mid-stream loads).
  - Stats for image 0 run on DVE (startup critical path); stats for images
    1-3 run 3-wide batched on the otherwise-idle GpSimd engine, entirely
    off the DVE queue.
  - b2 rides the combined tanh's per-partition bias (host pre-subtracts
    tile(b2,4) from W1's constant row); the DVE z2 path adds b2 during its
    PSUM read (tensor_scalar add); b3 rides the sigmoid bias.
  - PSUM (bank-granular dep tracking): 2 slots x 3 banks (slot = [z1 1024 |
    z2 256] at 0/1536), V/Vsq band sums in bank 6 strips, z3T/sigT blocks
    in bank 7.
  - Startup rings: image-0 im2col head split across sync/scalar HWDGE
    rings; b2s/w2s lead the scalar ring (tanh_0/MM2_0 need them); im2col
    tail gated (WAW on its own first columns) on bc(0,0) and spread over
    sync/scalar/pool rings.
"""

import numpy as np
import ml_dtypes

import concourse.bacc as bacc
import concourse.bass as bass
import concourse.tile as tile
import concourse.mybir as mybir
from concourse.bass_utils import run_bass_kernel_spmd

N_CORES = 8
B = 32
H = 107
PATCH = 11
R = 97          # output rows/cols
L = R * R       # 9409 positions per image
K = PATCH * PATCH  # 121
IPC = B // N_CORES  # images per core = 4
LP = L + 1      # 9410 (last tile even)
NT = 256        # positions per tile
TPI = (LP + NT - 1) // NT   # 37 tiles per image (36x256 + 194)
NTL = LP - 36 * NT          # 194, last tile width
G = IPC * TPI               # 148 tiles total
BCW = 6 * NT                # 1536, inv-broadcast group width

ZW = 128        # z2 columns offloaded to DVE (right half of z2)
Z1W = 4 * NT    # 1024, z1 region width in a slot
SW = Z1W + NT - ZW  # 1152, ScalarE tanh width per tile
Z2RB = 3072     # bank-6 base for z2 right halves (own bank: the DVE pair
                # read must not share a bank with tanh's reads, or the
                # bank-granular PSUM tracking serializes tanh behind it)
VB = (1280, 2816)  # V/Vsq strip bases (bank-2/5 spare, startup only)
Z3B = 3584      # bank-7 base for z3T/sigT blocks (2 x 128 cols)
NHEAD = 22      # image-0 im2col head rows

# clamped odd-quintic tanh: tanh(x) ~ clip(x*(A0 + A1 s + A2 s^2), -1, 1)
A0 = 0.9406599114084172
A1 = -0.18591269902220103
A2 = 0.0172626393963261
MAGIC = 0x5F3759DF
SQK = float(np.sqrt(K - 1))

F32 = mybir.dt.float32
BF16 = mybir.dt.bfloat16
I32 = mybir.dt.int32
Tanh = mybir.ActivationFunctionType.Tanh
Sigmoid = mybir.ActivationFunctionType.Sigmoid
Alu = mybir.AluOpType


def build():
    nc = bacc.Bacc("TRN2", target_bir_lowering=False, debug=False,
                   num_devices=N_CORES)
    x4 = nc.dram_tensor("x4", (IPC, H, H), BF16, kind="ExternalInput")
    w1e = nc.dram_tensor("w1e", (123, 512), BF16, kind="ExternalInput")
    w2t = nc.dram_tensor("w2t", (128, 512), BF16, kind="ExternalInput")
    b2c = nc.dram_tensor("b2c", (128, 1), F32, kind="ExternalInput")
    w3t = nc.dram_tensor("w3t", (128, 1), BF16, kind="ExternalInput")
    b3c = nc.dram_tensor("b3c", (1, 1), F32, kind="ExternalInput")
    av = nc.dram_tensor("av", (H, R), BF16, kind="ExternalInput")
    idm = nc.dram_tensor("idm", (128, 128), F32, kind="ExternalInput")
    y4 = nc.dram_tensor("y4", (IPC, L), F32, kind="ExternalOutput")
    invflat = nc.dram_tensor("invflat", (IPC, L), BF16, kind="Internal")

    xt = x4.ap().tensor
    invt_d = invflat.ap().tensor
    y4t = y4.ap().tensor

    tiles = [(i, t * NT, NT if t < TPI - 1 else NTL)
             for i in range(IPC) for t in range(TPI)]

    with tile.TileContext(nc) as tc:
        with (
            tc.tile_pool(name="wp", bufs=1) as wp,
            tc.tile_pool(name="stat", bufs=1) as st,
            tc.tile_pool(name="pim", bufs=3) as pim,
            tc.tile_pool(name="bcp", bufs=2) as bcp,
            tc.tile_pool(name="rhp", bufs=4) as rhp,
            tc.tile_pool(name="hp", bufs=6) as hp,
            tc.tile_pool(name="hxp", bufs=3) as hxp,
            tc.tile_pool(name="qp", bufs=2) as qp,
            tc.tile_pool(name="sgp", bufs=2) as sgp,
            tc.tile_pool(name="pg", bufs=1, space="PSUM") as pg,
        ):
            P = pg.tile([128, 4096], F32, tag="P")
            PS = P.ap[0][0]  # partition stride

            def slot(g):
                return (g % 2) * 1536

            def emit_im2col(img, i0=0, ni=R, mode="pool", alloc=True,
                            gate=None):
                if alloc:
                    pimgs[img] = pim.tile([123, LP], BF16, tag="pimg",
                                          name=f"pimg{img}")
                    nc.vector.memset(pimgs[img][:, L:LP], 0.0)
                pimg = pimgs[img]
                if gate is not None:
                    # WAW gate: a 2-col write inside the region these DMAs
                    # overwrite delays them until `gate` is produced.
                    nc.vector.tensor_copy(
                        pimg[0:123, i0 * R:i0 * R + 2], gate[0:123, 0:2])
                if mode == "head":
                    engs = [nc.gpsimd, nc.sync]
                elif mode == "tail":
                    engs = [nc.sync]
                else:
                    engs = [nc.gpsimd]
                for kh in range(PATCH):
                    eng = engs[kh % len(engs)]
                    eng.dma_start(
                        out=pimg[kh * PATCH:(kh + 1) * PATCH,
                                 i0 * R:(i0 + ni) * R]
                            .rearrange("p (i j) -> p i j", i=ni),
                        in_=bass.AP(tensor=xt,
                                    offset=img * H * H + (kh + i0) * H,
                                    ap=[[1, PATCH], [H, ni], [1, R]]))
                return pimg

            def emit_rows(img, eng=None):
                # pimg rows 121 (mean) and 122 (std) from the ms tile
                pimg = pimgs[img]
                for r in (0, 1):
                    (eng or nc.sync).dma_start(
                        out=pimg[121 + r:122 + r, 0:L]
                            .rearrange("p (i j) -> p i j", i=R),
                        in_=ms[:, img, r, :])

            def emit_bc_alloc(img):
                bc = bcp.tile([123, LP], BF16, tag="bci", name=f"bci{img}")
                nc.vector.memset(bc[:, L:LP], 1.0)  # pad position
                bcis[img] = bc
                return bc

            def emit_inv_store(img):
                # flatten the inv tile to DRAM row-major (broadcast source)
                invb, off, nimg = invbs[img]
                in_ = bass.AP(tensor=invb.tensor,
                              offset=invb.offset + off * R,
                              ap=[[nimg * R, R], [1, R]])
                nc.sync.dma_start(
                    out=bass.AP(tensor=invt_d, offset=img * L,
                                ap=[[1, 1], [1, L]]),
                    in_=in_)

            def emit_bc_chunk(img, r0, nr):
                # broadcast positions [97*r0, 97*(r0+nr)) to 123 partitions
                bc = bcis[img]
                nc.sync.dma_start(
                    out=bc[:, r0 * R:(r0 + nr) * R],
                    in_=bass.AP(tensor=invt_d, offset=img * L + r0 * R,
                                ap=[[0, 123], [1, nr * R]]))

            def emit_mmv(img):
                # V/Vsq band sums into bank-2/5 spare strips (img-2/3 reuse)
                base = VB[img % 2]
                nc.tensor.matmul(P[0:R, base:base + H],
                                 lhsT=avs, rhs=xall[:, img, :],
                                 start=True, stop=True)
                nc.tensor.matmul(P[0:R, base + H:base + 2 * H],
                                 lhsT=avs, rhs=xsq[:, img, :],
                                 start=True, stop=True)

            def emit_copyv(img):
                base = VB[img % 2]
                nc.vector.tensor_copy(
                    vv[:, img, :, :],
                    bass.AP(tensor=P.tensor, offset=P.offset + base,
                            ap=[[PS, R], [H, 2], [1, H]]))

            def quake_chain(ve, u, pfx, nimg):
                # inv = sqrt(K-1)*rsqrt(u) via bit-trick + 1 Newton step
                w = nimg * R
                ti = st.tile([R, w], I32, tag=f"{pfx}ti")
                ve.tensor_scalar(out=ti, in0=u[:].bitcast(I32), scalar1=1,
                                 scalar2=None, op0=Alu.logical_shift_right)
                yi = st.tile([R, w], I32, tag=f"{pfx}yi")
                ve.tensor_scalar(out=yi, in0=ti, scalar1=-1, scalar2=MAGIC,
                                 op0=Alu.mult, op1=Alu.add)
                y0 = yi[:].bitcast(F32)
                aa = st.tile([R, w], F32, tag=f"{pfx}aa")
                ve.tensor_mul(aa, y0, y0)
                bb = st.tile([R, w], F32, tag=f"{pfx}bb")
                ve.tensor_mul(bb, aa, u)
                cc = st.tile([R, w], F32, tag=f"{pfx}cc")
                ve.tensor_scalar(out=cc, in0=bb, scalar1=-0.5 * SQK,
                                 scalar2=1.5 * SQK, op0=Alu.mult, op1=Alu.add)
                y1f = st.tile([R, w], F32, tag=f"{pfx}y1f")
                ve.tensor_mul(y1f, y0, cc)
                invb = st.tile([R, w], BF16, tag=f"{pfx}invb")
                ve.tensor_copy(invb, y1f)
                return y1f, invb

            def emit_stats0():
                # image-0 stats on DVE (startup critical path)
                ve = nc.vector
                def vseg(o, w):
                    return bass.AP(tensor=vv.tensor, offset=vv.offset + o,
                                   ap=[vv.ap[0], [H, 2], [1, w]])
                w2v = st.tile([R, 2, H - 1], F32, tag="w2v")
                ve.tensor_add(w2v, vseg(0, H - 1), vseg(1, H - 1))
                w4v = st.tile([R, 2, H - 3], F32, tag="w4v")
                ve.tensor_add(w4v, w2v[:, :, 0:H - 3], w2v[:, :, 2:H - 1])
                w8v = st.tile([R, 2, H - 7], F32, tag="w8v")
                ve.tensor_add(w8v, w4v[:, :, 0:H - 7], w4v[:, :, 4:H - 3])
                tvv = st.tile([R, 2, R], F32, tag="tvv")
                ve.tensor_add(tvv, w8v[:, :, 0:R], w2v[:, :, 8:8 + R])
                sv = st.tile([R, 2, R], F32, tag="sv")
                ve.tensor_add(sv, tvv, vseg(10, R))
                t1 = st.tile([R, R], F32, tag="t1")
                ve.tensor_mul(t1, sv[:, 0, :], sv[:, 0, :])
                u = st.tile([R, R], F32, tag="u")
                ve.scalar_tensor_tensor(
                    out=u, in0=t1, scalar=-1.0 / K, in1=sv[:, 1, :],
                    op0=Alu.mult, op1=Alu.add)
                y1f, invb = quake_chain(ve, u, "q0", 1)
                ve.scalar_tensor_tensor(
                    out=ms[:, 0, 1, :], in0=u, scalar=1.0 / (K - 1),
                    in1=y1f, op0=Alu.mult, op1=Alu.mult)
                ve.tensor_scalar_mul(ms[:, 0, 0, :], sv[:, 0, :], 1.0 / K)
                invbs[0] = (invb, 0, 1)
                return invb

            def emit_stats_pool_adds():
                # images 1-3 batched: sliding sums + S^2 on GpSimd
                # (Pool supports only TensorTensor/Memset/AffineSelect)
                ve = nc.gpsimd
                def vseg(o, w):
                    return bass.AP(tensor=vv.tensor,
                                   offset=vv.offset + 2 * H + o,
                                   ap=[vv.ap[0], [H, 6], [1, w]])
                w2v = st.tile([R, 6, H - 1], F32, tag="bw2v")
                ve.tensor_add(w2v, vseg(0, H - 1), vseg(1, H - 1))
                w4v = st.tile([R, 6, H - 3], F32, tag="bw4v")
                ve.tensor_add(w4v, w2v[:, :, 0:H - 3], w2v[:, :, 2:H - 1])
                w8v = st.tile([R, 6, H - 7], F32, tag="bw8v")
                ve.tensor_add(w8v, w4v[:, :, 0:H - 7], w4v[:, :, 4:H - 3])
                tvv = st.tile([R, 6, R], F32, tag="btvv")
                ve.tensor_add(tvv, w8v[:, :, 0:R], w2v[:, :, 8:8 + R])
                sv = st.tile([R, 6, R], F32, tag="bsv")
                ve.tensor_add(sv, tvv, vseg(10, R))
                t1 = st.tile([R, 3, R], F32, tag="bt1")
                ve.tensor_mul(t1, sv[:, 0::2, :], sv[:, 0::2, :])
                return sv, t1

            def emit_stats_dve_step(k):
                # one step of the images-1-3 stats finish on DVE (spread
                # over otherwise-empty even loop iters)
                ve = nc.vector
                sv, t1 = bstat["svt"]
                if k == 0:
                    bstat["u"] = st.tile([R, 3, R], F32, tag="bu", name="bu")
                    ve.scalar_tensor_tensor(
                        out=bstat["u"], in0=t1, scalar=-1.0 / K,
                        in1=sv[:, 1::2, :], op0=Alu.mult, op1=Alu.add)
                elif k == 1:
                    bstat["ti"] = st.tile([R, 3 * R], I32, tag="bti", name="bti")
                    ve.tensor_scalar(out=bstat["ti"],
                                     in0=bstat["u"][:].bitcast(I32),
                                     scalar1=1, scalar2=None,
                                     op0=Alu.logical_shift_right)
                elif k == 2:
                    bstat["yi"] = st.tile([R, 3 * R], I32, tag="byi", name="byi")
                    ve.tensor_scalar(out=bstat["yi"], in0=bstat["ti"],
                                     scalar1=-1, scalar2=MAGIC,
                                     op0=Alu.mult, op1=Alu.add)
                elif k == 3:
                    bstat["aa"] = st.tile([R, 3 * R], F32, tag="baa", name="baa")
                    y0 = bstat["yi"][:].bitcast(F32)
                    ve.tensor_mul(bstat["aa"], y0, y0)
                elif k == 4:
                    bstat["bb"] = st.tile([R, 3, R], F32, tag="bbb", name="bbb")
                    ve.tensor_mul(bstat["bb"],
                                  bstat["aa"][:]
                                  .rearrange("p (i n) -> p i n", i=3),
                                  bstat["u"])
                elif k == 5:
                    bstat["cc"] = st.tile([R, 3 * R], F32, tag="bcc", name="bcc")
                    ve.tensor_scalar(
                        out=bstat["cc"],
                        in0=bstat["bb"][:].rearrange("p i n -> p (i n)"),
                        scalar1=-0.5 * SQK, scalar2=1.5 * SQK,
                        op0=Alu.mult, op1=Alu.add)
                elif k == 6:
                    bstat["y1f"] = st.tile([R, 3, R], F32, tag="by1f", name="by1f")
                    y0 = bstat["yi"][:].bitcast(F32)
                    ve.tensor_mul(bstat["y1f"][:]
                                  .rearrange("p i n -> p (i n)"),
                                  y0, bstat["cc"])
                elif k == 7:
                    invb = st.tile([R, 3 * R], BF16, tag="binvb", name="binvb")
                    ve.tensor_copy(invb, bstat["y1f"][:]
                                   .rearrange("p i n -> p (i n)"))
                    for i in (1, 2, 3):
                        invbs[i] = (invb, i - 1, 3)
                        emit_inv_store(i)
                elif k == 8:
                    ve.scalar_tensor_tensor(
                        out=ms[:, 1:4, 1, :], in0=bstat["u"],
                        scalar=1.0 / (K - 1), in1=bstat["y1f"],
                        op0=Alu.mult, op1=Alu.mult)
                elif k == 9:
                    ve.tensor_scalar_mul(ms[:, 1:4, 0, :], sv[:, 0::2, :],
                                         1.0 / K)

            # ---- startup ----
            xall = st.tile([H, IPC, H], BF16, tag="xall")
            nc.sync.dma_start(
                out=xall,
                in_=bass.AP(tensor=xt, offset=0,
                            ap=[[H, H], [H * H, IPC], [1, H]]))
            avs = wp.tile([H, R], BF16, tag="avs")
            nc.sync.dma_start(out=avs, in_=av.ap()[:, :])
            w1s = wp.tile([123, 512], BF16, tag="w1s")
            nc.sync.dma_start(out=w1s, in_=w1e.ap()[:, :])
            # b2s/w2s lead the scalar ring: tanh_0 / MM2_0 need them early
            b2s = wp.tile([128, 1], F32, tag="b2s")
            nc.scalar.dma_start(out=b2s, in_=b2c.ap()[:, :])
            w2s = wp.tile([128, 512], BF16, tag="w2s")
            nc.scalar.dma_start(out=w2s, in_=w2t.ap()[:, :])
            w3s = wp.tile([128, 1], BF16, tag="w3s")
            nc.scalar.dma_start(out=w3s, in_=w3t.ap()[:, :])
            b3s = wp.tile([128, 1], F32, tag="b3s")
            nc.scalar.dma_start(
                out=b3s,
                in_=bass.AP(tensor=b3c.ap().tensor, offset=0,
                            ap=[[0, 128], [1, 1]]))
            onesr = wp.tile([1, NT], BF16, tag="onesr")
            nc.vector.memset(onesr, 1.0)
            # dummy sigmoid: preload the (tanh+sigmoid) activation table now
            dummy = wp.tile([1, 1], BF16, tag="dummy")
            nc.scalar.activation(out=dummy, in_=onesr[0:1, 0:1],
                                 func=Sigmoid, bias=0.0)
            pimgs = {}
            emit_im2col(0, 0, NHEAD, mode="head")

            xsq = st.tile([H, IPC, H], BF16, tag="xsq")
            nc.vector.tensor_mul(xsq, xall, xall)
            ms = st.tile([R, IPC, 2, R], BF16, tag="ms")
            vv = st.tile([R, IPC, 2, H], F32, tag="vv")

            # zero z3T blocks once (junk cols/partials read by sigmoid)
            nc.vector.memset(P[0:128, Z3B:Z3B + 256], 0.0)

            bcis = {}
            invbs = {}
            emit_mmv(0)
            emit_mmv(1)
            emit_copyv(0)
            invb0 = emit_stats0()
            emit_copyv(1)
            emit_mmv(2)
            emit_copyv(2)
            emit_mmv(3)
            emit_copyv(3)
            emit_inv_store(0)
            emit_rows(0)
            emit_bc_alloc(0)
            emit_bc_chunk(0, 0, 3)
            emit_bc_chunk(0, 3, 32)
            hs = {}
            h2L = {}
            hxs = {}
            rhss = {}

            def emit_rhs(g):
                img, n0, nt = tiles[g]
                rhs = rhp.tile([123, NT], BF16, tag="rhs", name=f"rhs{g}")
                nc.vector.tensor_mul(rhs[:, 0:nt],
                                     pimgs[img][:, n0:n0 + nt],
                                     bcis[img][:, n0:n0 + nt])
                return rhs

            def emit_mm1(g, rhs):
                img, n0, nt = tiles[g]
                b = slot(g)
                for c in range(4):
                    nc.tensor.matmul(
                        P[:, b + NT * c:b + NT * c + nt],
                        lhsT=w1s[:, c * 128:(c + 1) * 128],
                        rhs=rhs[:, 0:nt], start=True, stop=True)

            def emit_tanh(g):
                img, n0, nt = tiles[g]
                b = slot(g)
                has_z2 = g >= 2
                h = hp.tile([128, SW], BF16, tag="h", name=f"h{g}")
                hs[g] = h
                if has_z2:
                    h2L[g - 2] = h[:, Z1W:SW]
                if nt == NT:
                    w = SW if has_z2 else Z1W
                    nc.scalar.activation(out=h[:, 0:w], in_=P[:, b:b + w],
                                         func=Tanh, bias=b2s[:, 0:1])
                else:
                    nc.scalar.activation(
                        out=h[:, 0:Z1W]
                            .rearrange("p (c n) -> p c n", c=4)[:, :, 0:nt],
                        in_=P[:, b:b + Z1W]
                            .rearrange("p (c n) -> p c n", c=4)[:, :, 0:nt],
                        func=Tanh, bias=b2s[:, 0:1])
                    if has_z2:
                        nc.scalar.activation(out=h[:, Z1W:SW],
                                             in_=P[:, b + Z1W:b + SW],
                                             func=Tanh, bias=b2s[:, 0:1])

            def emit_mm2(g):
                # z2 split: left half (tanh, ScalarE) into the slot; right
                # half (DVE quintic) into its own bank at Z2RB so the pair
                # read never shares a bank with tanh's reads
                img, n0, nt = tiles[g]
                bL = slot(g + 2) + Z1W
                bR = Z2RB + ZW * (g % 2)
                h = hs[g]
                wr = nt - (NT - ZW)
                for c in range(4):
                    nc.tensor.matmul(
                        P[:, bL:bL + NT - ZW],
                        lhsT=w2s[:, c * 128:(c + 1) * 128],
                        rhs=h[:, NT * c:NT * c + NT - ZW],
                        start=(c == 0), stop=(c == 3))
                for c in range(4):
                    nc.tensor.matmul(
                        P[:, bR:bR + wr],
                        lhsT=w2s[:, c * 128:(c + 1) * 128],
                        rhs=h[:, NT * c + NT - ZW:NT * c + nt],
                        start=(c == 0), stop=(c == 3))
                if wr != ZW:
                    # zero the junk z2R tail so the pair chain reads finite
                    nc.vector.memset(P[:, bR + wr:bR + ZW], 0.0)

            def emit_pair(k):
                # quintic tanh for z2 right halves of tiles 2k, 2k+1:
                # xc = z2R + b2; h = clip(xc*(A0 + A1 s + A2 s^2), -1, 1)
                xc = qp.tile([128, 2 * ZW], BF16, tag="xc", name=f"xc{k}")
                nc.vector.tensor_scalar(
                    out=xc, in0=P[:, Z2RB:Z2RB + 2 * ZW],
                    scalar1=b2s[:, 0:1], scalar2=None, op0=Alu.add)
                s = qp.tile([128, 2 * ZW], BF16, tag="s", name=f"s{k}")
                nc.vector.tensor_mul(s, xc, xc)
                q1 = qp.tile([128, 2 * ZW], BF16, tag="q1", name=f"q1{k}")
                nc.vector.tensor_scalar(out=q1, in0=s, scalar1=A2, scalar2=A1,
                                        op0=Alu.mult, op1=Alu.add)
                m1 = qp.tile([128, 2 * ZW], BF16, tag="m1", name=f"m1{k}")
                nc.vector.tensor_mul(m1, q1, s)
                q2 = qp.tile([128, 2 * ZW], BF16, tag="q2", name=f"q2{k}")
                nc.vector.tensor_scalar(out=q2, in0=m1, scalar1=A0,
                                        scalar2=None, op0=Alu.add)
                h0 = qp.tile([128, 2 * ZW], BF16, tag="h0", name=f"h0{k}")
                nc.vector.tensor_mul(h0, q2, xc)
                hx = hxp.tile([128, 2 * ZW], BF16, tag="hx", name=f"hx{k}")
                nc.vector.tensor_scalar(out=hx, in0=h0, scalar1=1.0,
                                        scalar2=-1.0, op0=Alu.min,
                                        op1=Alu.max)
                hxs[k] = hx

            def emit_mm3t(g2):
                img, n0, nt2 = tiles[g2]
                t2 = n0 // NT
                zc = Z3B + 128 * (img % 2) + 2 * t2
                nc.tensor.matmul(P[0:128, zc:zc + 1], lhsT=h2L[g2],
                                 rhs=w3s, start=True, stop=True)
                wd = nt2 - 128
                hx = hxs[g2 // 2]
                cb = ZW * (g2 % 2)
                nc.tensor.matmul(P[0:wd, zc + 1:zc + 2],
                                 lhsT=hx[:, cb:cb + wd],
                                 rhs=w3s, start=True, stop=True)

            def emit_finalize(img):
                blk = Z3B + 128 * (img % 2)
                sigS = sgp.tile([128, 74], F32, tag="sigS",
                                name=f"sigS{img}")
                nc.scalar.activation(out=sigS, in_=P[0:128, blk:blk + 74],
                                     func=Sigmoid, bias=b3s[:, 0:1])
                nc.tensor.transpose(P[0:74, blk:blk + 128], sigS, ident)
                sigT = sgp.tile([74, 128], F32, tag="sigT",
                                name=f"sigT{img}")
                nc.vector.tensor_copy(sigT, P[0:74, blk:blk + 128])
                nc.sync.dma_start(
                    out=bass.AP(tensor=y4t, offset=img * L,
                                ap=[[128, 73], [1, 128]]),
                    in_=sigT[0:73, :])
                nc.sync.dma_start(
                    out=bass.AP(tensor=y4t, offset=img * L + 73 * 128,
                                ap=[[1, 1], [1, L - 73 * 128]]),
                    in_=sigT[73:74, 0:L - 73 * 128])

            # prologue: rhs + MM1 for tiles 0-2
            rhss[0] = emit_rhs(0)
            rhss[1] = emit_rhs(1)
            rhss[2] = emit_rhs(2)
            emit_mm1(0, rhss[0])
            emit_mm1(1, rhss[1])

            # identity for the sigmoid transpose (built on DVE: Pool
            # does not support TensorScalarPtr-class ops in the real ISA)
            ident = wp.tile([128, 128], F32, tag="ident")
            nc.scalar.dma_start(out=ident, in_=idm.ap()[:, :])
            # stats for images 1-3: sums on GpSimd; finish spread over
            # even loop iterations on the DVE (see emit_stats_dve_step)
            bstat = {"svt": emit_stats_pool_adds()}
            # image-0 im2col tail: sync ring only, behind the startup DMAs
            emit_im2col(0, NHEAD, R - NHEAD, mode="tail", alloc=False)
            emit_bc_chunk(0, 35, 32)
            emit_bc_chunk(0, 67, 30)

            for g in range(G):
                img, n0, nt = tiles[g]
                t = n0 // NT
                # prefetch im2col + mean/std rows for next image mid-stream
                if g % 2 == 0 and 6 <= g <= 24:
                    emit_stats_dve_step((g - 6) // 2)
                if t == 10 and img + 1 < IPC:
                    # gate on the current tile's h so the scheduler cannot
                    # hoist these bulk DMAs into earlier device windows
                    emit_im2col(img + 1, gate=hs[g - 1])
                if t == 26 and img + 1 < IPC:
                    emit_rows(img + 1)
                # prefetch next image's inv-broadcast buffer in chunks
                if img + 1 < IPC:
                    if t == 24:
                        emit_bc_alloc(img + 1)
                        emit_bc_chunk(img + 1, 0, 25)
                    elif t == 26:
                        emit_bc_chunk(img + 1, 25, 24)
                    elif t == 28:
                        emit_bc_chunk(img + 1, 49, 24)
                    elif t == 30:
                        emit_bc_chunk(img + 1, 73, 24)
                # rhs ahead of the pair chain so MM1 never waits the chain
                if g % 2 == 1:
                    if g + 2 < G:
                        rhss[g + 2] = emit_rhs(g + 2)
                    if g + 3 < G:
                        rhss[g + 3] = emit_rhs(g + 3)
                if g >= 41 and (g - 41) % TPI == 0 and (g - 41) // TPI < 3:
                    emit_finalize((g - 41) // TPI)
                # mm3t first in the PE block: its deps are iters old, so it
                # never queues behind MM1/MM2 (which wait on tanh_g)
                if g >= 4:
                    emit_mm3t(g - 4)
                emit_tanh(g)
                if g + 2 < G:
                    emit_mm1(g + 2, rhss[g + 2])
                emit_mm2(g)
                if g % 2 == 1:
                    emit_pair((g - 1) // 2)

            # epilogue: z2L of tiles G-2, G-1; last MM3Ts; finalize img 3
            for gg in (G, G + 1):
                b = slot(gg)
                hz = hp.tile([128, ZW], BF16, tag="hz", name=f"hz{gg}")
                nc.scalar.activation(out=hz, in_=P[:, b + Z1W:b + SW],
                                     func=Tanh, bias=b2s[:, 0:1])
                h2L[gg - 2] = hz[:, 0:ZW]
            for g2 in (G - 4, G - 3, G - 2, G - 1):
                emit_mm3t(g2)
            emit_finalize(IPC - 1)
    nc.compile()
    return nc


def prep_inputs(x, W1, b1, W2, b2, W3, b3):
    x = np.asarray(x, dtype=np.float32)
    W1 = np.asarray(W1, dtype=np.float32)
    b1 = np.asarray(b1, dtype=np.float32)
    W2 = np.asarray(W2, dtype=np.float32)
    b2 = np.asarray(b2, dtype=np.float32)
    W3 = np.asarray(W3, dtype=np.float32)
    b3 = np.asarray(b3, dtype=np.float32)
    bf = ml_dtypes.bfloat16

    Wp = W1[:, 1:]  # (512, 121)
    w1e = np.concatenate(
        [Wp.T, -Wp.sum(axis=1)[None, :],
         (W1[:, 0] + b1 - np.tile(b2, 4))[None, :]],
        axis=0).astype(bf)  # (123, 512)
    w2t = np.concatenate(
        [W2[:, c * 128:(c + 1) * 128].T for c in range(4)],
        axis=1).astype(bf)  # (128, 512)
    b2c = b2[:, None].astype(np.float32).copy()  # (128, 1)
    w3t = W3.T.astype(bf).copy()  # (128, 1)
    b3c = b3.reshape(1, 1).astype(np.float32).copy()
    av = np.zeros((H, R), dtype=np.float32)
    for i in range(R):
        av[i:i + PATCH, i] = 1.0
    av = av.astype(bf)

    shared = {"w1e": w1e, "w2t": w2t, "b2c": b2c, "w3t": w3t,
              "b3c": b3c, "av": av,
              "idm": np.eye(128, dtype=np.float32)}
    in_maps = []
    for c in range(N_CORES):
        m = dict(shared)
        m["x4"] = np.ascontiguousarray(x[c * IPC:(c + 1) * IPC, 0]).astype(bf)
        in_maps.append(m)
    return in_maps


_CACHE = {}


def kernel(x, W1, b1, W2, b2, W3, b3):
    nc = _CACHE.get("nc")
    if nc is None:
        nc = build(**_CACHE.get("build_kwargs", {}))
        _CACHE["nc"] = nc
    in_maps = prep_inputs(x, W1, b1, W2, b2, W3, b3)
    res = run_bass_kernel_spmd(nc, in_maps, core_ids=list(range(N_CORES)))
    y = np.stack([res.results[c]["y4"] for c in range(N_CORES)])  # (8,4,L)
    return y.reshape(B, 1, R, R).astype(np.float32)


if __name__ == "__main__":
    rng = np.random.default_rng(0)
    inputs = {
        "x": rng.standard_normal((B, 1, H, H), dtype=np.float32),
        "W1": (rng.standard_normal((512, 122)) * 0.05).astype(np.float32),
        "b1": (rng.standard_normal((512,)) * 0.05).astype(np.float32),
        "W2": (rng.standard_normal((128, 512)) * 0.05).astype(np.float32),
        "b2": (rng.standard_normal((128,)) * 0.05).astype(np.float32),
        "W3": (rng.standard_normal((1, 128)) * 0.05).astype(np.float32),
        "b3": (rng.standard_normal((1,)) * 0.05).astype(np.float32),
    }
    out = kernel(**inputs)
    print(out.shape, out.dtype)
